# revision 10
# baseline (speedup 1.0000x reference)
"""Trainium2 Bass kernel for nn_LocalFeatureGuided.

Pipeline per image (C=128 on partitions, spatial on free dim):
  BN(eval)+GELU (ACT, fused affine, both column parities in one op) ->
    zero-padded even/odd column-parity planes in fp8e4m3
  depthwise 7x7 s2 conv, split by output rows:
    rows [0,NR_DVE) on DVE (scalar_tensor_tensor FMA, two interleaved
    accumulation chains, fp32 acc);
    rows [NR_DVE,64) on PE as fp8 diag-matmuls with DoubleRow perf mode
    (parity planes as the pair dim; kw=6 taps pair rows kh/kh+2) ->
    25 passes per 8-row chunk.
  k matmuls (all 5 tokens) are uniform: PE matmul [C,1024] -> psum ->
    copy to SBUF bf16 (alternating ACT/DVE) -> dot vs q0:
    m>=1 dots on GpSimd (Pool) stt with accum_out, m=0 dots on DVE.
    The x-token (m>=1) matmuls chase the input DMA and fill the PE
    idle head before gelu/conv are ready.
  softmax over 5 logits per (b,c), v & proj fused as in the baseline:
    out = sum_m (Wv^T diag(a_m) Pw^T)^T @ t_m.
Emission is a single global schedule interleaving both images so each
engine's in-order stream has its dependencies produced just in time.
Sharding: data-parallel over batch, 2 images per core, 8 cores.
"""

import os
import numpy as np
from contextlib import ExitStack

import concourse.bass as bass
import concourse.tile as tile
from concourse import bacc, mybir
from concourse import bass_utils
from concourse import tile_utils

alu = mybir.AluOpType
actf = mybir.ActivationFunctionType
F32 = mybir.dt.float32
BF16 = mybir.dt.bfloat16
FP8 = mybir.dt.float8e4

B, C, H, W = 16, 128, 128, 128
H2, W2 = H // 2, W // 2
L = H2 * W2            # 4096
NCORES = 8
BPC = B // NCORES      # 2 images per core
EPS = 1e-5
INV_SQRT_C = 1.0 / np.sqrt(128.0)

# ---- tuning knobs ----
NR_DVE = int(os.environ.get("NR_DVE", "8"))   # conv rows [0,NR_DVE) on DVE
# m>=1 dots on GpSimd — walrus rejects TensorScalarPtr on Pool, keep 0
POOL_DOTS = int(os.environ.get("POOL_DOTS", "0"))
SBUF_CAP = int(os.environ.get("SBUF_CAP", str(206 * 1024)))

TAPS = [(kh, kw) for kh in range(7) for kw in range(7)]


def build(nc):
    x_d = nc.dram_tensor("x", (BPC, C, H, W), BF16, kind="ExternalInput").ap()
    bns_d = nc.dram_tensor("bns", (C, 1), F32, kind="ExternalInput").ap()
    bnb_d = nc.dram_tensor("bnb", (C, 1), F32, kind="ExternalInput").ap()
    dww_d = nc.dram_tensor("dww", (C, 49), F32, kind="ExternalInput").ap()
    dwb_d = nc.dram_tensor("dwb", (C, 1), F32, kind="ExternalInput").ap()
    diag_d = nc.dram_tensor("diag", (C, 49 * 128), FP8,
                            kind="ExternalInput").ap()
    csc_d = nc.dram_tensor("cscale", (C, 1), F32, kind="ExternalInput").ap()
    wqT_d = nc.dram_tensor("wqT", (C, C), BF16, kind="ExternalInput").ap()
    wkT_d = nc.dram_tensor("wkT", (C, C), BF16, kind="ExternalInput").ap()
    wv_d = nc.dram_tensor("wv", (C, C), BF16, kind="ExternalInput").ap()
    pwT_d = nc.dram_tensor("pwT", (C, C), F32, kind="ExternalInput").ap()
    bq_d = nc.dram_tensor("bq", (C, 1), F32, kind="ExternalInput").ap()
    bk_d = nc.dram_tensor("bk", (C, 1), F32, kind="ExternalInput").ap()
    cb_d = nc.dram_tensor("cb", (C, 1), F32, kind="ExternalInput").ap()
    out_d = nc.dram_tensor("out", (BPC, C, H2, W2), F32,
                           kind="ExternalOutput").ap()

    with tile.TileContext(nc) as tc, ExitStack() as ctx:
        tp = lambda name, bufs, **kw: ctx.enter_context(
            tc.tile_pool(name=name, bufs=bufs, **kw))

        wpool = tp("weights", 1)       # persistent small weights
        xp = tp("x", 2)
        gp = tp("gelu", 2)
        t0p = tp("t0", 2)
        q0p = tp("q0", 2)
        accp = tp("acc", 2)
        outp = tp("outc", 3)
        vecp = tp("vec", 16)
        emp = tp("em", 2)
        kcp = tp("kc", 14)             # k chunks copied to SBUF bf16
        scrp = tp("scr", 2)            # DVE dot scratch
        pscrp = tp("pscr", 2)          # Pool dot scratch
        vtp = tp("vt", 4)
        ppk = tp("ppk", 2, space="PSUM")    # [C,1024] = 2 banks each
        pp512 = tp("pp512", 4, space="PSUM")  # [C,512] = 1 bank each

        # ---------- persistent weights ----------
        def vec_load(src_ap, eng=None):
            t = vecp.tile([C, 1], F32, tag="v")
            (eng or nc.gpsimd).dma_start(t[:], src_ap)
            return t

        bns = vec_load(bns_d, nc.sync)
        bnb = vec_load(bnb_d, nc.sync)
        # prime the ACT Gelu table so the first gelu band doesn't pay it
        warm = vecp.tile([C, 1], F32, tag="v")
        nc.scalar.activation(warm[:], bns[:], actf.Gelu)
        # eo pad memsets on gpsimd (idle at start)
        eo_bufs = []
        for i in range(BPC):
            eo = gp.tile([C, 2, 134, 68], FP8, tag="eo")
            eng = nc.gpsimd
            eng.memset(eo[:, :, 0:3], 0.0)
            eng.memset(eo[:, :, 131:134], 0.0)
            eng.memset(eo[:, 0, 3:131, 0:1], 0.0)
            eng.memset(eo[:, 0, 3:131, 65:68], 0.0)
            eng.memset(eo[:, 1, 3:131, 0:2], 0.0)
            eng.memset(eo[:, 1, 3:131, 66:68], 0.0)
            eo_bufs.append(eo)
        dwb = vec_load(dwb_d)
        csc = vec_load(csc_d)
        bq = vec_load(bq_d)
        bk = vec_load(bk_d)
        cb = vec_load(cb_d)

        dww = wpool.tile([C, 49], F32)
        nc.gpsimd.dma_start(dww[:], dww_d)
        diag = wpool.tile([C, 49 * 128], FP8)
        for s in range(8):  # off the x stream, via the gpsimd queue
            sl = slice(s * 784, (s + 1) * 784)
            nc.gpsimd.dma_start(diag[:, sl], diag_d[:, sl])
        wqT = wpool.tile([C, C], BF16)
        nc.gpsimd.dma_start(wqT[:], wqT_d)
        wkT = wpool.tile([C, C], BF16)
        nc.sync.dma_start(wkT[:], wkT_d)
        wv = wpool.tile([C, C], BF16)
        nc.gpsimd.dma_start(wv[:], wv_d)
        pwT = wpool.tile([C, C], F32)
        nc.gpsimd.dma_start(pwT[:], pwT_d)

        # ---------- per-image persistent tiles ----------
        xs = []
        for img in range(BPC):
            st = dict(img=img)
            st["x_t"] = xp.tile([C, H * W], BF16, name=f"xt{img}", tag="xt")
            st["x3"] = st["x_t"][:].rearrange("c (h w) -> c h w", h=H)
            st["eo"] = eo_bufs[img]
            st["t0"] = t0p.tile([C, L], BF16, name=f"t0_{img}", tag="t0")
            st["q0"] = q0p.tile([C, L], BF16, name=f"q0_{img}", tag="q0")
            st["q0sums"] = vecp.tile([C, 4], F32, tag="q0s", name=f"q0s{img}")
            st["dots"] = vecp.tile([C, 20], F32, tag="dots", name=f"dots{img}")
            st["kc"] = {}
            st["oi"] = out_d[img].rearrange("c h w -> c (h w)")
            st["xi"] = x_d[img].rearrange("c h w -> c (h w)")
            xs.append(st)

        # ---------- emission helpers ----------
        def x_dma(st, s0, s1):
            # slice s covers x rows [8s, 8s+8) = cols [1024s, 1024(s+1))
            for s in range(s0, s1):
                nc.sync.dma_start(st["x_t"][:, s * 1024:(s + 1) * 1024],
                                  st["xi"][:, s * 1024:(s + 1) * 1024])

        def gelu_band(st, bnd):
            # one merged op writes both parity planes:
            #   E[3+r, 1+j] = g[r, 2j], O[3+r, 2+j] = g[r, 2j+1]
            r0, r1 = 16 * bnd, 16 * (bnd + 1)
            eo = st["eo"]
            out = eo[:, 0:2, 3 + r0:3 + r1, 1:65]
            out.ap[1] = [134 * 68 + 1, 2]   # odd plane starts one col later
            inp = st["x3"][:, r0:r1, 0::2].unsqueeze(1)
            inp.ap[1] = [1, 2]              # parity axis: x col +1
            nc.scalar.activation(out, inp, actf.Gelu,
                                 bias=bnb[:, 0:1], scale=bns[:, 0:1])

        def tok_ap(st, m, c0, c1):
            if m == 0:
                return st["t0"][:, c0:c1]
            p, q = (m - 1) // 2, (m - 1) % 2
            assert c0 % 64 == 0 and c1 % 64 == 0
            return st["x3"][:, p::2, q::2][:, c0 // 64:c1 // 64, :]

        # k matmul + SBUF copy; early copies alternate ACT/DVE to split
        # load, the conv-phase ones stay on ACT (DVE is deep in conv then)
        kcop_flip = [0]

        def k_mm(st, m, hf, cop="alt"):
            kp = ppk.tile([C, 1024], F32, tag="kp")
            for j in range(2):  # matmul out must fit one PSUM bank (512 f32)
                c0 = hf * 1024 + j * 512
                nc.tensor.matmul(kp[:, j * 512:(j + 1) * 512], wkT[:],
                                 tok_ap(st, m, c0, c0 + 512),
                                 start=True, stop=True)
            kc = kcp.tile([C, 1024], BF16, tag="kc")
            if cop == "alt" and kcop_flip[0] % 2 == 1:
                nc.vector.tensor_copy(kc[:], kp[:])
            else:
                nc.scalar.copy(kc[:], kp[:])
            kcop_flip[0] += 1
            st["kc"][(m, hf)] = kc

        def dot(st, m, hf):
            kc = st["kc"].pop((m, hf))
            q0s = st["q0"][:, hf * 1024:(hf + 1) * 1024]
            dcol = st["dots"][:, m * 4 + hf:m * 4 + hf + 1]
            if m >= 1 and POOL_DOTS:
                scr = pscrp.tile([C, 1024], BF16, tag="ps")
                nc.gpsimd.scalar_tensor_tensor(
                    scr[:], q0s, 1.0, kc[:], alu.mult, alu.mult,
                    accum_out=dcol)
            else:
                scr = scrp.tile([C, 1024], BF16, tag="s")
                nc.vector.scalar_tensor_tensor(
                    scr[:], q0s, 1.0, kc[:], alu.mult, alu.mult,
                    accum_out=dcol)

        def q0_mm(st, hf):
            ps = ppk.tile([C, 1024], F32, tag="kp")
            for j in range(2):
                c0 = hf * 1024 + j * 512
                nc.tensor.matmul(ps[:, j * 512:(j + 1) * 512], wqT[:],
                                 st["t0"][:, c0:c0 + 512],
                                 start=True, stop=True)
            nc.scalar.activation(st["q0"][:, hf * 1024:(hf + 1) * 1024],
                                 ps[:], actf.Identity, bias=bq[:, 0:1],
                                 accum_out=st["q0sums"][:, hf:hf + 1])

        # ---- conv helpers (same structure as the proven baseline) ----
        def g_ap(st, kh, kw, a, b):
            e = kw - 3
            par, u = (0, e // 2) if e % 2 == 0 else (1, (e - 1) // 2)
            off = (1 + u) if par == 0 else (2 + u)
            return st["eo"][:, par, kh + 2 * a:kh + 2 * b:2, off:off + 64]

        def conv_dve(st):
            # rows [0, NR_DVE): two independent DVE accumulation chains.
            # The ACT copy into t0 is emitted separately (t0dve action) so
            # it doesn't head-block the in-order ACT queue for ~30us.
            acc0 = accp.tile([C, NR_DVE, 64], F32, tag="a0")
            acc1 = accp.tile([C, NR_DVE, 64], F32, tag="a1")
            accs = [acc0, acc1]
            for i, (kh, kw) in enumerate(TAPS):
                w_s = dww[:, kh * 7 + kw:kh * 7 + kw + 1]
                acc = accs[i % 2]
                if i < 2:
                    nc.vector.tensor_scalar_mul(
                        acc[:], g_ap(st, kh, kw, 0, NR_DVE), w_s)
                else:
                    nc.vector.scalar_tensor_tensor(
                        acc[:], g_ap(st, kh, kw, 0, NR_DVE), w_s, acc[:],
                        alu.mult, alu.add)
            nc.vector.tensor_tensor(accs[0][:], accs[0][:], accs[1][:],
                                    alu.add)
            st["dve_acc"] = accs[0]

        def t0_dve_cop(st):
            nc.scalar.activation(
                st["t0"][:, :NR_DVE * 64],
                st["dve_acc"][:].rearrange("c h w -> c (h w)"),
                actf.Identity, bias=dwb[:, 0:1])

        def conv_chunk(st, r0, r1):
            # PE part: fp8 DoubleRow diag matmuls, 25 passes per 8 rows
            eo = st["eo"]
            nrw = r1 - r0
            ps = pp512.tile([C, nrw * 64], F32, tag="cv")
            for p in range(21):  # pair index: kh = p // 3, j = p % 3
                kh, j = p // 3, p % 3
                lhsT = diag[:, p * 256:(p + 1) * 256].rearrange(
                    "c (two f) -> c two f", two=2)
                rhs = eo[:, :, kh + 2 * r0:kh + 2 * r1:2, j:j + 64]
                nc.tensor.matmul(ps[:], lhsT, rhs, start=(p == 0),
                                 stop=False,
                                 perf_mode=mybir.MatmulPerfMode.DoubleRow)
            for pi, kh in enumerate((0, 1, 4)):
                # kw=6 taps paired (kh, kh+2) via overlapping-stride pair dim
                lhsT = diag[:, (42 + 2 * pi) * 128:
                            (44 + 2 * pi) * 128].rearrange(
                    "c (two f) -> c two f", two=2)
                rhs = eo[:, 1, kh + 2 * r0:kh + 2 * r1:2, 3:67]
                rhs = rhs.unsqueeze(1)
                rhs.ap[1] = [136, 2]  # second plane = rows +2 (tap kh+2)
                nc.tensor.matmul(ps[:], lhsT, rhs, start=False, stop=False,
                                 perf_mode=mybir.MatmulPerfMode.DoubleRow)
            nc.tensor.matmul(  # lone tap (kh=5, kw=6)
                ps[:], diag[:, 48 * 128:49 * 128],
                eo[:, 1, 5 + 2 * r0:5 + 2 * r1:2, 3:67],
                start=False, stop=True)
            nc.scalar.activation(st["t0"][:, r0 * 64:r1 * 64], ps[:],
                                 actf.Identity, bias=dwb[:, 0:1],
                                 scale=csc[:, 0:1])

        def softmax(st):
            dots, q0sums = st["dots"], st["q0sums"]
            s5 = vecp.tile([C, 5], F32, tag="s5")
            nc.vector.tensor_reduce(
                s5[:], dots[:].rearrange("c (m h) -> c m h", m=5),
                mybir.AxisListType.X, alu.add)
            q0s = vecp.tile([C, 1], F32, tag="v")
            nc.vector.tensor_reduce(q0s[:], q0sums[:], mybir.AxisListType.X,
                                    alu.add)
            bkqs = vecp.tile([C, 1], F32, tag="v")
            nc.vector.scalar_tensor_tensor(bkqs[:], bk[:], INV_SQRT_C,
                                           q0s[:], alu.mult, alu.mult)
            e5 = vecp.tile([C, 5], F32, tag="s5")
            nc.scalar.activation(e5[:], s5[:], actf.Exp, bias=bkqs[:, 0:1],
                                 scale=INV_SQRT_C)
            ssum = vecp.tile([C, 1], F32, tag="v")
            nc.vector.tensor_reduce(ssum[:], e5[:], mybir.AxisListType.X,
                                    alu.add)
            sinv = vecp.tile([C, 1], F32, tag="v")
            nc.vector.reciprocal(sinv[:], ssum[:])
            a5 = vecp.tile([C, 5], F32, tag="s5")
            nc.vector.tensor_scalar_mul(a5[:], e5[:], sinv[:, 0:1])

            # fused v+proj weights: lhsT_m = Wv^T diag(a_m) Pw^T
            em_all = emp.tile([C, 5, C], BF16, tag="em")
            nc.vector.tensor_tensor(
                em_all[:],
                pwT[:].rearrange("c (m d) -> c m d", m=1).broadcast_to(
                    (C, 5, C)),
                a5[:].rearrange("c (m o) -> c m o", o=1).broadcast_to(
                    (C, 5, C)),
                alu.mult)
            emf = em_all[:].rearrange("c m d -> c (m d)")
            vt_all = vtp.tile([C, 5 * C], BF16, tag="vt")
            for lo, hi in ((0, 4), (4, 5)):
                vp = pp512.tile([C, 512], F32, tag="cv")
                nc.tensor.matmul(vp[:, :(hi - lo) * C], wv[:],
                                 emf[:, lo * C:hi * C], start=True,
                                 stop=True)
                nc.scalar.copy(vt_all[:, lo * C:hi * C],
                               vp[:, :(hi - lo) * C])
            st["vt"] = vt_all

        def v_chunk(st, ch):
            vt_all = st["vt"]
            ps = pp512.tile([C, 512], F32, tag="cv")
            for m in range(5):
                nc.tensor.matmul(
                    ps[:], vt_all[:, m * C:(m + 1) * C],
                    tok_ap(st, m, ch * 512, (ch + 1) * 512),
                    start=(m == 0), stop=(m == 4))
            oc = outp.tile([C, 512], F32, tag="oc")
            nc.scalar.activation(oc[:], ps[:], actf.Identity,
                                 bias=cb[:, 0:1])
            nc.sync.dma_start(st["oi"][:, ch * 512:(ch + 1) * 512], oc[:])

        # ================= global schedule =================
        s0, s1 = xs[0], xs[1]
        # input DMA: img0 first, img1 trickles behind
        x_dma(s0, 0, 6)
        x_dma(s1, 0, 2)
        x_dma(s0, 6, 16)
        x_dma(s1, 2, 16)

        # img0 gelu + early k matmuls (x tokens hf0..hf2), woven by x
        # arrival; hf3's are deferred into the conv phase so the kc pool
        # never holds more than ~13 undotted chunks (14 bufs).
        gelu_band(s0, 0)
        gelu_band(s0, 1)
        for m in (1, 2, 3, 4):
            k_mm(s0, m, 0)
        for hf in (1, 2):
            gelu_band(s0, 2 * hf)
            for m in (1, 2):
                k_mm(s0, m, hf)
            gelu_band(s0, 2 * hf + 1)
            for m in (3, 4):
                k_mm(s0, m, hf)
        gelu_band(s0, 6)
        gelu_band(s0, 7)

        chunks = [(r, min(r + 8, 64)) for r in range(NR_DVE, 64, 8)]
        nch = len(chunks)
        assert nch == 7, "post schedule assumes NR_DVE=8"

        # per-chunk actions: q0/k0 chase t0 availability; hf0 needs the
        # DVE rows (ready late), so it lands with hf3 at the end.
        post = {i: [] for i in range(nch)}
        post[2] += [("q0", 1), ("k0", 1)]
        post[4] += [("q0", 2), ("k0", 2)]
        post[5] += [("kE3", None), ("t0dve", None)]
        post[6] += [("q0", 0), ("k0", 0), ("q0", 3), ("k0", 3)]

        def run_post(st, acts):
            for kind, hf in acts:
                if kind == "q0":
                    q0_mm(st, hf)
                    for m in (1, 2, 3, 4):
                        dot(st, m, hf)
                elif kind == "k0":
                    k_mm(st, 0, hf, cop="act")
                    dot(st, 0, hf)
                elif kind == "kE3":
                    for m in (1, 2, 3, 4):
                        k_mm(st, m, 3, cop="act")
                elif kind == "t0dve":
                    t0_dve_cop(st)

        # img0 conv: DVE rows + PE chunks; gelu1 bands woven between
        conv_dve(s0)
        for ci, (r0, r1) in enumerate(chunks):
            conv_chunk(s0, r0, r1)
            if ci < 8:
                gelu_band(s1, ci)
            run_post(s0, post[ci])
        gelu_band(s1, 7)

        # img0 softmax/vt while img1's early k matmuls start on PE
        softmax(s0)
        kq1 = [(m, hf) for hf in (0, 1, 2) for m in (1, 2, 3, 4)]
        for m, hf in kq1[:2]:
            k_mm(s1, m, hf)
        kq1 = kq1[2:]
        # img0 v phase with img1 k matmuls woven in
        for ch in range(8):
            v_chunk(s0, ch)
            for _ in range(2):
                if kq1:
                    k_mm(s1, *kq1.pop(0))
        while kq1:
            k_mm(s1, *kq1.pop(0))

        # img1 conv + q0/k0/dots
        conv_dve(s1)
        for ci, (r0, r1) in enumerate(chunks):
            conv_chunk(s1, r0, r1)
            run_post(s1, post[ci])

        softmax(s1)
        for ch in range(8):
            v_chunk(s1, ch)
    return nc


_CACHE = {}


def _get_nc():
    if "nc" not in _CACHE:
        tile_utils.max_sbuf_usage = SBUF_CAP
        nc = bacc.Bacc("TRN2", target_bir_lowering=False, debug=False,
                       num_devices=NCORES)
        build(nc)
        nc.compile()
        _CACHE["nc"] = nc
    return _CACHE["nc"]


def _in_maps(x, bn_gamma, bn_beta, bn_mean, bn_var, dw_w, dw_b, qkv_w, qkv_b,
             proj_w, proj_b):
    import ml_dtypes
    bf16 = ml_dtypes.bfloat16
    f32 = np.float32
    bn_gamma = np.asarray(bn_gamma, f32)
    bn_beta = np.asarray(bn_beta, f32)
    bn_mean = np.asarray(bn_mean, f32)
    bn_var = np.asarray(bn_var, f32)
    dw_w = np.asarray(dw_w, f32).reshape(C, 49)
    dw_b = np.asarray(dw_b, f32)
    qkv_w = np.asarray(qkv_w, f32)
    qkv_b = np.asarray(qkv_b, f32)
    proj_w = np.asarray(proj_w, f32)
    proj_b = np.asarray(proj_b, f32)

    bns = bn_gamma / np.sqrt(bn_var + np.float32(EPS))
    bnb = bn_beta - bn_mean * bns

    # fp8 conv weights, pre-scaled per channel by a power of two so they
    # sit in e4m3's normal range; the t0 copy descales via ACT's scale.
    wmax = np.abs(dw_w).max(axis=1)
    S = np.exp2(np.floor(np.log2(128.0 / np.maximum(wmax, 1e-30)))).astype(f32)
    cscale = (1.0 / S).reshape(C, 1)
    # block order: 21 pairs [(kh, 2j+1) even-plane, (kh, 2j) odd-plane],
    # then kw=6 row-pairs [(kh,6),(kh+2,6)] for kh in (0,1,4), then (5,6)
    diag = np.zeros((C, 49 * 128), f32)
    blocks = []
    for kh in range(7):
        for j in range(3):
            blocks.append((kh, 2 * j + 1))
            blocks.append((kh, 2 * j))
    for kh in (0, 1, 4):
        blocks.append((kh, 6))
        blocks.append((kh + 2, 6))
    blocks.append((5, 6))
    for bi, (kh, kw) in enumerate(blocks):
        diag[np.arange(C), bi * 128 + np.arange(C)] = dw_w[:, kh * 7 + kw] * S

    bv = qkv_b[2 * C:3 * C]
    fp8 = ml_dtypes.float8_e4m3fn
    shared = {
        "bns": bns.reshape(C, 1),
        "bnb": bnb.reshape(C, 1),
        "dww": np.ascontiguousarray(dw_w),
        "dwb": dw_b.reshape(C, 1),
        "diag": diag.astype(fp8),
        "cscale": cscale,
        "wqT": np.ascontiguousarray(qkv_w[0:C].T).astype(bf16),
        "wkT": np.ascontiguousarray(qkv_w[C:2 * C].T).astype(bf16),
        "wv": np.ascontiguousarray(qkv_w[2 * C:3 * C]).astype(bf16),
        "pwT": np.ascontiguousarray(proj_w.T),
        "bq": qkv_b[0:C].reshape(C, 1),
        "bk": qkv_b[C:2 * C].reshape(C, 1),
        "cb": (proj_w @ bv + proj_b).reshape(C, 1),
    }
    xf = np.ascontiguousarray(np.asarray(x, f32)).astype(bf16)
    return [dict(shared, x=xf[i * BPC:(i + 1) * BPC]) for i in range(NCORES)]


def kernel(**inputs):
    nc = _get_nc()
    res = bass_utils.run_bass_kernel_spmd(nc, _in_maps(**inputs),
                                          core_ids=list(range(NCORES)))
    return np.concatenate([r["out"] for r in res.results], axis=0)
